# revision 32
# baseline (speedup 1.0000x reference)
"""CrossViewConLoss Trainium2 kernel (8 NeuronCores, SPMD, symmetric-half).

Math: features (2048, 3, 512) -> F = permute/reshape to (6144, 512);
Fn = row-normalized F; sim = Fn @ Fn.T (6144 x 6144, symmetric);
num_i = sum_{j in block(i)} exp(sim_ij)   (3 blocks of 2048 rows)
den_i = sum_j exp(|sim_ij|)
loss = -(sum_i log(num_i / den_i)) / 2048

sim is symmetric, so each element is computed ONCE and credited to both
its row (row-sum via ACT accum) and its column (column-sum via Pool
partition-reduce or a ones-matmul).  Work is split per core c with every
2048-row block's rows rotated by 256*c (host-side roll), which makes all
8 cores run the IDENTICAL instruction stream on different data:

  Part A (in-block): each block is a ring of 16 column tiles.  Local row
  tile i (i=0,1 per block) processes column tiles i..i+8: d'=0 (diag,
  row-sums only), d'=1..7 (row+col sums), d'=8 (row-sums only, the pair
  is half-counted from both sides).  Both exp(sim) (numerator) and
  exp(|sim|) (denominator) row/col sums are produced.
  Part B (off-block): block pairs (0,1),(1,2),(2,0): rows = this core's
  2 row tiles of b1, columns = all of b2.  exp(|sim|) row sums via ACT
  accum, column sums via ones-matmuls accumulated in PSUM.

Host: normalizes F (the sharding_hint shards "the normalized feature
matrix"), builds per-core rotated layouts, and does the final all-reduce:
scatter-adds the 8 cores' partial row/col sums into global num/den then
loss = -(sum log(num/den))/batch in float64.

Engine balance per core (cost model): PE ~37us (matmuls + B col-sums),
ACT ~31us (all exp passes + row accums), Pool ~26us (|.| from PSUM +
A col-sum partition-reduces), DVE ~1us (d8 row reduces), DMA ~21us.
"""

import sys

import numpy as np

_TRN_REPO = "/opt/trn_rl_repo"
if _TRN_REPO not in sys.path:
    sys.path.insert(0, _TRN_REPO)

import concourse.bacc as bacc
import concourse.mybir as mybir
import concourse.tile as tile
from concourse.bass_utils import run_bass_kernel_spmd

N_CORES = 8
BATCH, VIEW, DIM = 2048, 3, 512
N = BATCH * VIEW            # 6144 rows
KT = DIM // 128             # 4 contraction tiles
ROT = 256                   # per-core row rotation within each block
PAIRS = ((2, 0), (0, 1), (1, 2))
DT = mybir.dt.float16
F8 = mybir.dt.float8e4
F32 = mybir.dt.float32
I32 = mybir.dt.int32
DR = mybir.MatmulPerfMode.DoubleRow
SCALE = 8.0            # Fn values are scaled by 8 before fp8 cast
ISCL = 1.0 / (SCALE * SCALE)   # folded into the exp activations
A = mybir.AluOpType
AF = mybir.ActivationFunctionType
AX = mybir.AxisListType

_cache = {}


def _build_nc():
    nc = bacc.Bacc("TRN2", debug=False, num_devices=N_CORES)
    fnt_d = nc.dram_tensor("fnt", [128, 2, 2, N], F8, kind="ExternalInput")
    rowsT_d = nc.dram_tensor("rowsT", [128, 2, 2, 768], F8, kind="ExternalInput")
    rowout_d = nc.dram_tensor("rowout", [128, 36], F32, kind="ExternalOutput")
    cs_d = nc.dram_tensor("cs", [128, 180], F32, kind="ExternalOutput")

    with tile.TileContext(nc) as tc:
        _emit_body(nc, tc, fnt_d, rowsT_d, rowout_d, cs_d)
    nc.compile()
    return nc


def _emit_body(nc, tc, fnt_d, rowsT_d, rowout_d, cs_d):
    with (
        tc.tile_pool(name="singles", bufs=1) as singles,
        tc.tile_pool(name="big_pool", bufs=16) as big_pool,
        tc.tile_pool(name="abw_pool", bufs=3) as abw_pool,
        tc.tile_pool(name="pscs", bufs=1, space="PSUM") as pscs,
        tc.tile_pool(name="psmm", bufs=3, space="PSUM") as psmm,
    ):
        fnt = singles.tile([128, 2, 2, N], F8, name="fnt")
        rt = singles.tile([128, 2, 2, 768], F8, name="rt")
        rowacc = singles.tile([128, 36], F32, name="rowacc")
        csout = singles.tile([128, 180], F32, name="csout")
        ones = singles.tile([128, 1], DT, name="ones")
        # p8 (the six half-counted 128-col tiles) and the column-sum
        # accumulators share one 2-bank PSUM pool; the matmul pool gets 3
        # double-bank tiles.
        shared = pscs.tile([128, 948], F32, name="shared")
        p8 = shared[:, 0:768]
        csP = shared[:, 768:948]
        csA = shared[:, 768:852]
        csB = shared[:, 852:948]

        nc.vector.memset(ones[:], 1.0)
        nc.gpsimd.memset(rowacc[:], 0.0)
        warm = singles.tile([128, 1], DT, name="warm")
        nc.scalar.activation(warm[:], ones[:], AF.Exp)
        # DMA order: row-tile-0 weights; block-0 A-window k-split (the first
        # tile can start after its k=0 slice); remaining weights; then
        # window/remainder chunks in the order the interleaved schedule
        # consumes them.  BORDER below matches: B pair (2,0) runs first.
        nc.sync.dma_start(rt[:, :, :, 0:128], rowsT_d[:, :, :, 0:128])
        for k2 in range(2):
            nc.sync.dma_start(fnt[:, k2, :, 0:1280], fnt_d[:, k2, :, 0:1280])
        nc.sync.dma_start(rt[:, :, :, 128:768], rowsT_d[:, :, :, 128:768])
        nc.sync.dma_start(fnt[:, :, :, 1280:2048], fnt_d[:, :, :, 1280:2048])
        for b in (1, 2):
            c0 = BATCH * b
            nc.sync.dma_start(fnt[:, :, :, c0:c0 + 1280],
                              fnt_d[:, :, :, c0:c0 + 1280])
            nc.sync.dma_start(fnt[:, :, :, c0 + 1280:c0 + 2048],
                              fnt_d[:, :, :, c0 + 1280:c0 + 2048])

        pending = []

        def flush(keep=0):
            while len(pending) > keep:
                pending.pop(0)()

        def emit_a_split():
            # first tile runs as two 512-col halves so ACT starts ~1.4us
            # earlier (only the first 640-col DMA slices are needed);
            # row-sums land in the spare d8 cells (6/12 num, 18/24 den).
            for h in range(2):
                P = psmm.tile([128, 1024], F32, tag="mm", name="P")
                for k2 in range(2):
                    nc.tensor.matmul(
                        P[:, 0:512],
                        rt[:, k2, :, 0:128],
                        fnt[:, k2, :, 512 * h:512 * h + 512],
                        start=(k2 == 0), stop=(k2 == 1), perf_mode=DR)
                ep = big_pool.tile([128, 512], DT, tag="bigh", name="eph")
                nc.scalar.activation(ep[:], P[:, 0:512], AF.Exp, scale=ISCL,
                                     accum_out=rowacc[:, 32 * h:32 * h + 1])
                en = big_pool.tile([128, 512], DT, tag="bigh", name="enh")
                nc.scalar.activation(en[:], P[:, 0:512], AF.Exp, scale=-ISCL)
                eb = big_pool.tile([128, 512], DT, tag="bigh", name="ebh")
                nc.vector.scalar_tensor_tensor(
                    eb[:], ep[:], 1.0, en[:], A.mult, A.max,
                    accum_out=rowacc[:, 12 + 21 * h:13 + 21 * h])

                def colsums(h=h, ep=ep, eb=eb):
                    lo = 128 if h == 0 else 0
                    for c in range(lo, 512, 128):
                        cc = (512 * h + c) // 128 - 1
                        nc.tensor.matmul(
                            csA[:, cc:cc + 1], ep[:, c:c + 128],
                            ones[:, 0:1], start=True, stop=True)
                        nc.tensor.matmul(
                            csA[:, 7 + cc:8 + cc], eb[:, c:c + 128],
                            ones[:, 0:1], start=True, stop=True)
                pending.append(colsums)
            for k2 in range(2):
                nc.tensor.matmul(
                    p8[:, 0:128], rt[:, k2, :, 0:128],
                    fnt[:, k2, :, 1024:1152],
                    start=(k2 == 0), stop=(k2 == 1), perf_mode=DR)

        def emit_a(b, i):
            m = 2 * b + i
            c0 = BATCH * b + 128 * i
            P = psmm.tile([128, 1024], F32, tag="mm", name="P")
            for k2 in range(2):
                for n0 in (0, 512):
                    nc.tensor.matmul(
                        P[:, n0:n0 + 512],
                        rt[:, k2, :, 128 * m:128 * (m + 1)],
                        fnt[:, k2, :, c0 + n0:c0 + n0 + 512],
                        start=(k2 == 0), stop=(k2 == 1), perf_mode=DR)
            c8 = BATCH * b + 128 * (i + 8)
            for k2 in range(2):
                nc.tensor.matmul(
                    p8[:, 128 * m:128 * (m + 1)],
                    rt[:, k2, :, 128 * m:128 * (m + 1)],
                    fnt[:, k2, :, c8:c8 + 128],
                    start=(k2 == 0), stop=(k2 == 1), perf_mode=DR)
            flush(keep=4)
            ep = big_pool.tile([128, 1024], DT, tag="big", name="ep")
            nc.scalar.activation(ep[:], P[:], AF.Exp, scale=ISCL,
                                 accum_out=rowacc[:, m:m + 1])
            en = big_pool.tile([128, 1024], DT, tag="big", name="en")
            if i == 1 and b > 0:
                # DVE-side exp(-s): balances ACT vs DVE across the kernel
                with nc.allow_low_precision(reason="exp(-s)=1/exp(s), fp16"):
                    nc.vector.reciprocal(en[:], ep[:])
            else:
                nc.scalar.activation(en[:], P[:], AF.Exp, scale=-ISCL)
            eb = big_pool.tile([128, 1024], DT, tag="big", name="eb")
            nc.vector.scalar_tensor_tensor(
                eb[:], ep[:], 1.0, en[:], A.mult, A.max,
                accum_out=rowacc[:, 12 + m:13 + m])

            def colsums(m=m, ep=ep, eb=eb):
                for c in range(7):
                    nc.tensor.matmul(
                        csA[:, 14 * m + c:14 * m + c + 1],
                        ep[:, 128 * (c + 1):128 * (c + 2)],
                        ones[:, 0:1], start=True, stop=True)
                for c in range(7):
                    nc.tensor.matmul(
                        csA[:, 14 * m + 7 + c:14 * m + 8 + c],
                        eb[:, 128 * (c + 1):128 * (c + 2)],
                        ones[:, 0:1], start=True, stop=True)
            pending.append(colsums)

        def emit_b(pi, i, split_tail=False):
            b1, b2 = PAIRS[pi]
            m = 2 * b1 + i
            halves = []
            ab = abw_pool.tile([128, 2048], I32, tag="abw", name="abB")
            for half in range(2):
                c0 = BATCH * b2 + 1024 * half
                P = psmm.tile([128, 1024], F32, tag="mm", name="PB")
                for k2 in range(2):
                    for n0 in (0, 512):
                        nc.tensor.matmul(
                            P[:, n0:n0 + 512],
                            rt[:, k2, :, 128 * m:128 * (m + 1)],
                            fnt[:, k2, :, c0 + n0:c0 + n0 + 512],
                            start=(k2 == 0), stop=(k2 == 1), perf_mode=DR)
                flush(keep=4)
                nc.vector.tensor_scalar(
                    ab[:, 1024 * half:1024 * (half + 1)],
                    P.bitcast(I32), 0x7FFFFFFF, None, A.bitwise_and)
                if split_tail:
                    eb = big_pool.tile([128, 1024], DT, tag="big", name="ebB")
                    cell = 30 + half
                    nc.scalar.activation(
                        eb[:, :],
                        ab.bitcast(F32)[:, 1024 * half:1024 * (half + 1)],
                        AF.Exp, scale=ISCL,
                        accum_out=rowacc[:, cell:cell + 1])
                    halves.append(eb)
            if not split_tail:
                eb = big_pool.tile([128, 2048], DT, tag="bigw", name="ebBw")
                cell = 24 + pi * 2 + i
                nc.scalar.activation(eb[:], ab.bitcast(F32), AF.Exp, scale=ISCL,
                                     accum_out=rowacc[:, cell:cell + 1])
                halves = [eb[:, 0:1024], eb[:, 1024:2048]]

            def colsums(idx=pi * 2 + i, halves=halves):
                for c in range(16):
                    nc.tensor.matmul(
                        csB[:, 16 * idx + c:16 * idx + c + 1],
                        halves[c // 8][:, 128 * (c % 8):128 * (c % 8 + 1)],
                        ones[:, 0:1], start=True, stop=True)
            pending.append(colsums)

        # interleaved schedule: each super-group = one block's two A tiles
        # followed by one B pair (whose PE-bound stretch drains the A
        # activation backlog).  B pair order matches the DMA order above.
        def emit_d8():
            # d8 elementwise (p8 accumulated during the A tiles)
            ep8 = big_pool.tile([128, 768], DT, tag="big", name="ep8")
            nc.scalar.activation(ep8[:], p8[:], AF.Exp, scale=ISCL)
            nc.vector.tensor_reduce(
                rowacc[:, 6:12], ep8.rearrange("p (m j) -> p m j", j=128),
                axis=AX.X, op=A.add)
            en8 = big_pool.tile([128, 768], DT, tag="big", name="en8")
            nc.scalar.activation(en8[:], p8[:], AF.Exp, scale=-ISCL)
            eb8 = big_pool.tile([128, 768], DT, tag="big", name="eb8")
            nc.vector.scalar_tensor_tensor(
                eb8[:], ep8[:], 1.0, en8[:], A.mult, A.max)
            nc.vector.tensor_reduce(
                rowacc[:, 18:24], eb8.rearrange("p (m j) -> p m j", j=128),
                axis=AX.X, op=A.add)

        BORDER = (0, 1, 2)   # pi into PAIRS = ((2,0),(0,1),(1,2))
        for g in range(3):
            emit_a(g, 0)
            emit_a(g, 1)
            if g == 2:
                emit_d8()
            pi = BORDER[g]
            emit_b(pi, 0)
            emit_b(pi, 1, split_tail=(g == 2))
        flush()

        nc.vector.tensor_copy(csout[:], csP[:])
        nc.sync.dma_start(rowout_d[:], rowacc[:])
        nc.sync.dma_start(cs_d[:], csout[:])


def _pack8(M):
    """[rows, 512] float32 -> DoubleRow plane layout [128, 2, 2, rows] fp8.

    Element (p, k2, q, j) holds M[j, 256*k2 + 128*q + p] * SCALE; the matmul
    pairs lhs/rhs by (p, q) so any consistent (p,q)->d mapping works.
    """
    import ml_dtypes
    a = (M.T * SCALE).reshape(2, 2, 128, M.shape[0])      # [k2, q, p, j]
    return np.ascontiguousarray(
        a.transpose(2, 0, 1, 3)).astype(ml_dtypes.float8_e4m3fn)


def _prep_inputs(features: np.ndarray):
    F = np.ascontiguousarray(
        features.transpose(1, 0, 2).reshape(N, DIM)).astype(np.float32)
    norms = np.maximum(np.sqrt((F * F).sum(-1, keepdims=True)), 1e-8)
    Fn = F / norms
    Fnb = Fn.reshape(VIEW, BATCH, DIM)
    in_maps = []
    for c in range(N_CORES):
        rot = [np.roll(Fnb[b], -ROT * c, axis=0) for b in range(VIEW)]
        fnt_local = np.concatenate(rot, axis=0)            # [6144, 512]
        rows = np.concatenate([r[0:256] for r in rot], axis=0)  # [768, 512]
        in_maps.append({
            "fnt": _pack8(fnt_local),
            "rowsT": _pack8(rows),
        })
    return in_maps


def _combine(results):
    num_g = np.zeros(N, dtype=np.float64)
    den_g = np.zeros(N, dtype=np.float64)
    ar = np.arange
    for c in range(N_CORES):
        rowout = results[c]["rowout"].astype(np.float64)
        cs = results[c]["cs"].astype(np.float64)
        for b in range(VIEW):
            for i in range(2):
                m = 2 * b + i
                g = BATCH * b + (ROT * c + 128 * i + ar(128)) % BATCH
                num_g[g] += rowout[:, m] + rowout[:, 6 + m]
                den_g[g] += rowout[:, 12 + m] + rowout[:, 18 + m]
                if b == 0 and i == 0:
                    num_g[g] += rowout[:, 32]
                    den_g[g] += rowout[:, 33]
                pi_of = {p[0]: j for j, p in enumerate(PAIRS)}
                pi = pi_of[b]
                if pi == 2 and 2 * PAIRS[pi][0] + i == 2 * PAIRS[2][0] + 1:
                    den_g[g] += rowout[:, 30] + rowout[:, 31]
                else:
                    den_g[g] += rowout[:, 24 + pi * 2 + i]
                for cc in range(7):
                    gc = BATCH * b + (ROT * c + 128 * (i + 1 + cc) + ar(128)) % BATCH
                    num_g[gc] += cs[:, 14 * m + cc]
                    den_g[gc] += cs[:, 14 * m + 7 + cc]
        for pi, (_b1, b2) in enumerate(PAIRS):
            for i in range(2):
                idx = pi * 2 + i
                for cc in range(16):
                    gc = BATCH * b2 + (ROT * c + 128 * cc + ar(128)) % BATCH
                    den_g[gc] += cs[:, 84 + 16 * idx + cc]
    loss = -(np.log(num_g / den_g).sum() / BATCH)
    return np.float32(loss)


def run(features: np.ndarray, trace: bool = False):
    """Run the SPMD kernel; returns (loss ndarray, BassKernelResults)."""
    if "nc" not in _cache:
        _cache["nc"] = _build_nc()
    nc = _cache["nc"]
    in_maps = _prep_inputs(np.asarray(features))
    res = run_bass_kernel_spmd(nc, in_maps, core_ids=list(range(N_CORES)),
                               trace=trace)
    loss = _combine(res.results)
    return np.asarray(loss, dtype=np.float32), res


def kernel(features: np.ndarray) -> np.ndarray:
    loss, _ = run(features, trace=False)
    return loss


# revision 33
# speedup vs baseline: 1.0095x; 1.0095x over previous
"""CrossViewConLoss Trainium2 kernel (8 NeuronCores, SPMD, symmetric-half).

Math: features (2048, 3, 512) -> F = permute/reshape to (6144, 512);
Fn = row-normalized F; sim = Fn @ Fn.T (6144 x 6144, symmetric);
num_i = sum_{j in block(i)} exp(sim_ij)   (3 blocks of 2048 rows)
den_i = sum_j exp(|sim_ij|)
loss = -(sum_i log(num_i / den_i)) / 2048

sim is symmetric, so each element is computed ONCE and credited to both
its row (row-sum via ACT accum) and its column (column-sum via Pool
partition-reduce or a ones-matmul).  Work is split per core c with every
2048-row block's rows rotated by 256*c (host-side roll), which makes all
8 cores run the IDENTICAL instruction stream on different data:

  Part A (in-block): each block is a ring of 16 column tiles.  Local row
  tile i (i=0,1 per block) processes column tiles i..i+8: d'=0 (diag,
  row-sums only), d'=1..7 (row+col sums), d'=8 (row-sums only, the pair
  is half-counted from both sides).  Both exp(sim) (numerator) and
  exp(|sim|) (denominator) row/col sums are produced.
  Part B (off-block): block pairs (0,1),(1,2),(2,0): rows = this core's
  2 row tiles of b1, columns = all of b2.  exp(|sim|) row sums via ACT
  accum, column sums via ones-matmuls accumulated in PSUM.

Host: normalizes F (the sharding_hint shards "the normalized feature
matrix"), builds per-core rotated layouts, and does the final all-reduce:
scatter-adds the 8 cores' partial row/col sums into global num/den then
loss = -(sum log(num/den))/batch in float64.

Engine balance per core (cost model): PE ~37us (matmuls + B col-sums),
ACT ~31us (all exp passes + row accums), Pool ~26us (|.| from PSUM +
A col-sum partition-reduces), DVE ~1us (d8 row reduces), DMA ~21us.
"""

import sys

import numpy as np

_TRN_REPO = "/opt/trn_rl_repo"
if _TRN_REPO not in sys.path:
    sys.path.insert(0, _TRN_REPO)

import concourse.bacc as bacc
import concourse.mybir as mybir
import concourse.tile as tile
from concourse.bass_utils import run_bass_kernel_spmd

N_CORES = 8
BATCH, VIEW, DIM = 2048, 3, 512
N = BATCH * VIEW            # 6144 rows
KT = DIM // 128             # 4 contraction tiles
ROT = 256                   # per-core row rotation within each block
PAIRS = ((2, 0), (0, 1), (1, 2))
DT = mybir.dt.float16
F8 = mybir.dt.float8e4
F32 = mybir.dt.float32
I32 = mybir.dt.int32
DR = mybir.MatmulPerfMode.DoubleRow
SCALE = 8.0            # Fn values are scaled by 8 before fp8 cast
ISCL = 1.0 / (SCALE * SCALE)   # folded into the exp activations
A = mybir.AluOpType
AF = mybir.ActivationFunctionType
AX = mybir.AxisListType

_cache = {}


def _build_nc():
    nc = bacc.Bacc("TRN2", debug=False, num_devices=N_CORES)
    fnt_d = nc.dram_tensor("fnt", [128, 2, 2, N], F8, kind="ExternalInput")
    rowsT_d = nc.dram_tensor("rowsT", [128, 2, 2, 768], F8, kind="ExternalInput")
    rowout_d = nc.dram_tensor("rowout", [128, 36], F32, kind="ExternalOutput")
    cs_d = nc.dram_tensor("cs", [128, 180], F32, kind="ExternalOutput")

    with tile.TileContext(nc) as tc:
        _emit_body(nc, tc, fnt_d, rowsT_d, rowout_d, cs_d)
    nc.compile()
    return nc


def _emit_body(nc, tc, fnt_d, rowsT_d, rowout_d, cs_d):
    with (
        tc.tile_pool(name="singles", bufs=1) as singles,
        tc.tile_pool(name="big_pool", bufs=16) as big_pool,
        tc.tile_pool(name="abw_pool", bufs=3) as abw_pool,
        tc.tile_pool(name="pscs", bufs=1, space="PSUM") as pscs,
        tc.tile_pool(name="psmm", bufs=3, space="PSUM") as psmm,
    ):
        fnt = singles.tile([128, 2, 2, N], F8, name="fnt")
        rt = singles.tile([128, 2, 2, 768], F8, name="rt")
        rowacc = singles.tile([128, 36], F32, name="rowacc")
        csout = singles.tile([128, 180], F32, name="csout")
        ones = singles.tile([128, 1], DT, name="ones")
        # p8 (the six half-counted 128-col tiles) and the column-sum
        # accumulators share one 2-bank PSUM pool; the matmul pool gets 3
        # double-bank tiles.
        shared = pscs.tile([128, 948], F32, name="shared")
        p8 = shared[:, 0:768]
        csP = shared[:, 768:948]
        csA = shared[:, 768:852]
        csB = shared[:, 852:948]

        nc.vector.memset(ones[:], 1.0)
        nc.gpsimd.memset(rowacc[:], 0.0)
        warm = singles.tile([128, 1], DT, name="warm")
        nc.scalar.activation(warm[:], ones[:], AF.Exp)
        # DMA order: row-tile-0 weights; block-0 A-window k-split (the first
        # tile can start after its k=0 slice); remaining weights; then
        # window/remainder chunks in the order the interleaved schedule
        # consumes them.  BORDER below matches: B pair (2,0) runs first.
        nc.sync.dma_start(rt[:, :, :, 0:128], rowsT_d[:, :, :, 0:128])
        for k2 in range(2):
            nc.sync.dma_start(fnt[:, k2, :, 0:1280], fnt_d[:, k2, :, 0:1280])
        nc.sync.dma_start(rt[:, :, :, 128:768], rowsT_d[:, :, :, 128:768])
        nc.sync.dma_start(fnt[:, :, :, 1280:2048], fnt_d[:, :, :, 1280:2048])
        for b in (1, 2):
            c0 = BATCH * b
            nc.sync.dma_start(fnt[:, :, :, c0:c0 + 1280],
                              fnt_d[:, :, :, c0:c0 + 1280])
            nc.sync.dma_start(fnt[:, :, :, c0 + 1280:c0 + 2048],
                              fnt_d[:, :, :, c0 + 1280:c0 + 2048])

        pending = []

        def flush(keep=0):
            while len(pending) > keep:
                pending.pop(0)()

        def emit_a_split():
            # first tile runs as two 512-col halves so ACT starts ~1.4us
            # earlier (only the first 640-col DMA slices are needed);
            # row-sums land in the spare d8 cells (6/12 num, 18/24 den).
            for h in range(2):
                P = psmm.tile([128, 1024], F32, tag="mm", name="P")
                for k2 in range(2):
                    nc.tensor.matmul(
                        P[:, 0:512],
                        rt[:, k2, :, 0:128],
                        fnt[:, k2, :, 512 * h:512 * h + 512],
                        start=(k2 == 0), stop=(k2 == 1), perf_mode=DR)
                ep = big_pool.tile([128, 512], DT, tag="bigh", name="eph")
                nc.scalar.activation(ep[:], P[:, 0:512], AF.Exp, scale=ISCL,
                                     accum_out=rowacc[:, 32 * h:32 * h + 1])
                en = big_pool.tile([128, 512], DT, tag="bigh", name="enh")
                nc.scalar.activation(en[:], P[:, 0:512], AF.Exp, scale=-ISCL)
                eb = big_pool.tile([128, 512], DT, tag="bigh", name="ebh")
                nc.vector.scalar_tensor_tensor(
                    eb[:], ep[:], 1.0, en[:], A.mult, A.max,
                    accum_out=rowacc[:, 12 + 21 * h:13 + 21 * h])

                def colsums(h=h, ep=ep, eb=eb):
                    lo = 128 if h == 0 else 0
                    for c in range(lo, 512, 128):
                        cc = (512 * h + c) // 128 - 1
                        nc.tensor.matmul(
                            csA[:, cc:cc + 1], ep[:, c:c + 128],
                            ones[:, 0:1], start=True, stop=True)
                        nc.tensor.matmul(
                            csA[:, 7 + cc:8 + cc], eb[:, c:c + 128],
                            ones[:, 0:1], start=True, stop=True)
                pending.append(colsums)
            for k2 in range(2):
                nc.tensor.matmul(
                    p8[:, 0:128], rt[:, k2, :, 0:128],
                    fnt[:, k2, :, 1024:1152],
                    start=(k2 == 0), stop=(k2 == 1), perf_mode=DR)

        def emit_a(b, i):
            m = 2 * b + i
            c0 = BATCH * b + 128 * i
            P = psmm.tile([128, 1024], F32, tag="mm", name="P")
            for k2 in range(2):
                for n0 in (0, 512):
                    nc.tensor.matmul(
                        P[:, n0:n0 + 512],
                        rt[:, k2, :, 128 * m:128 * (m + 1)],
                        fnt[:, k2, :, c0 + n0:c0 + n0 + 512],
                        start=(k2 == 0), stop=(k2 == 1), perf_mode=DR)
            c8 = BATCH * b + 128 * (i + 8)
            for k2 in range(2):
                nc.tensor.matmul(
                    p8[:, 128 * m:128 * (m + 1)],
                    rt[:, k2, :, 128 * m:128 * (m + 1)],
                    fnt[:, k2, :, c8:c8 + 128],
                    start=(k2 == 0), stop=(k2 == 1), perf_mode=DR)
            flush(keep=4)
            ep = big_pool.tile([128, 1024], DT, tag="big", name="ep")
            nc.scalar.activation(ep[:], P[:], AF.Exp, scale=ISCL,
                                 accum_out=rowacc[:, m:m + 1])
            en = big_pool.tile([128, 1024], DT, tag="big", name="en")
            if i == 1 and b > 0:
                # DVE-side exp(-s): balances ACT vs DVE across the kernel
                with nc.allow_low_precision(reason="exp(-s)=1/exp(s), fp16"):
                    nc.vector.reciprocal(en[:], ep[:])
            else:
                nc.scalar.activation(en[:], P[:], AF.Exp, scale=-ISCL)
            eb = big_pool.tile([128, 1024], DT, tag="big", name="eb")
            nc.vector.scalar_tensor_tensor(
                eb[:], ep[:], 1.0, en[:], A.mult, A.max,
                accum_out=rowacc[:, 12 + m:13 + m])

            def colsums(m=m, ep=ep, eb=eb):
                for c in range(7):
                    nc.tensor.matmul(
                        csA[:, 14 * m + c:14 * m + c + 1],
                        ep[:, 128 * (c + 1):128 * (c + 2)],
                        ones[:, 0:1], start=True, stop=True)
                for c in range(7):
                    nc.tensor.matmul(
                        csA[:, 14 * m + 7 + c:14 * m + 8 + c],
                        eb[:, 128 * (c + 1):128 * (c + 2)],
                        ones[:, 0:1], start=True, stop=True)
            pending.append(colsums)

        def emit_b(pi, i, split_tail=False):
            b1, b2 = PAIRS[pi]
            m = 2 * b1 + i
            halves = []
            ab = abw_pool.tile([128, 2048], I32, tag="abw", name="abB")
            for half in range(2):
                c0 = BATCH * b2 + 1024 * half
                P = psmm.tile([128, 1024], F32, tag="mm", name="PB")
                for k2 in range(2):
                    for n0 in (0, 512):
                        nc.tensor.matmul(
                            P[:, n0:n0 + 512],
                            rt[:, k2, :, 128 * m:128 * (m + 1)],
                            fnt[:, k2, :, c0 + n0:c0 + n0 + 512],
                            start=(k2 == 0), stop=(k2 == 1), perf_mode=DR)
                flush(keep=4)
                nc.vector.tensor_scalar(
                    ab[:, 1024 * half:1024 * (half + 1)],
                    P.bitcast(I32), 0x7FFFFFFF, None, A.bitwise_and)
                if split_tail:
                    eb = big_pool.tile([128, 1024], DT, tag="big", name="ebB")
                    cell = 30 + half
                    nc.scalar.activation(
                        eb[:, :],
                        ab.bitcast(F32)[:, 1024 * half:1024 * (half + 1)],
                        AF.Exp, scale=ISCL,
                        accum_out=rowacc[:, cell:cell + 1])
                    halves.append(eb)
            if not split_tail:
                eb = big_pool.tile([128, 2048], DT, tag="bigw", name="ebBw")
                cell = 24 + pi * 2 + i
                nc.scalar.activation(eb[:], ab.bitcast(F32), AF.Exp, scale=ISCL,
                                     accum_out=rowacc[:, cell:cell + 1])
                halves = [eb[:, 0:1024], eb[:, 1024:2048]]

            def colsums(idx=pi * 2 + i, halves=halves):
                for c in range(16):
                    nc.tensor.matmul(
                        csB[:, 16 * idx + c:16 * idx + c + 1],
                        halves[c // 8][:, 128 * (c % 8):128 * (c % 8 + 1)],
                        ones[:, 0:1], start=True, stop=True)
            pending.append(colsums)

        # interleaved schedule: each super-group = one block's two A tiles
        # followed by one B pair (whose PE-bound stretch drains the A
        # activation backlog).  B pair order matches the DMA order above.
        def emit_d8():
            # d8 elementwise (p8 accumulated during the A tiles)
            ep8 = big_pool.tile([128, 768], DT, tag="big", name="ep8")
            nc.scalar.activation(ep8[:], p8[:], AF.Exp, scale=ISCL)
            nc.vector.tensor_reduce(
                rowacc[:, 6:12], ep8.rearrange("p (m j) -> p m j", j=128),
                axis=AX.X, op=A.add)
            en8 = big_pool.tile([128, 768], DT, tag="big", name="en8")
            nc.scalar.activation(en8[:], p8[:], AF.Exp, scale=-ISCL)
            eb8 = big_pool.tile([128, 768], DT, tag="big", name="eb8")
            nc.vector.scalar_tensor_tensor(
                eb8[:], ep8[:], 1.0, en8[:], A.mult, A.max)
            nc.vector.tensor_reduce(
                rowacc[:, 18:24], eb8.rearrange("p (m j) -> p m j", j=128),
                axis=AX.X, op=A.add)

        BORDER = (0, 1, 2)   # pi into PAIRS = ((2,0),(0,1),(1,2))
        for g in range(3):
            pi = BORDER[g]
            emit_a(g, 0)
            emit_b(pi, 0)
            emit_a(g, 1)
            if g == 2:
                emit_d8()
            emit_b(pi, 1, split_tail=(g == 2))
        flush()

        nc.vector.tensor_copy(csout[:], csP[:])
        nc.sync.dma_start(rowout_d[:], rowacc[:])
        nc.sync.dma_start(cs_d[:], csout[:])


def _pack8(M):
    """[rows, 512] float32 -> DoubleRow plane layout [128, 2, 2, rows] fp8.

    Element (p, k2, q, j) holds M[j, 256*k2 + 128*q + p] * SCALE; the matmul
    pairs lhs/rhs by (p, q) so any consistent (p,q)->d mapping works.
    """
    import ml_dtypes
    a = (M.T * SCALE).reshape(2, 2, 128, M.shape[0])      # [k2, q, p, j]
    return np.ascontiguousarray(
        a.transpose(2, 0, 1, 3)).astype(ml_dtypes.float8_e4m3fn)


def _prep_inputs(features: np.ndarray):
    F = np.ascontiguousarray(
        features.transpose(1, 0, 2).reshape(N, DIM)).astype(np.float32)
    norms = np.maximum(np.sqrt((F * F).sum(-1, keepdims=True)), 1e-8)
    Fn = F / norms
    Fnb = Fn.reshape(VIEW, BATCH, DIM)
    in_maps = []
    for c in range(N_CORES):
        rot = [np.roll(Fnb[b], -ROT * c, axis=0) for b in range(VIEW)]
        fnt_local = np.concatenate(rot, axis=0)            # [6144, 512]
        rows = np.concatenate([r[0:256] for r in rot], axis=0)  # [768, 512]
        in_maps.append({
            "fnt": _pack8(fnt_local),
            "rowsT": _pack8(rows),
        })
    return in_maps


def _combine(results):
    num_g = np.zeros(N, dtype=np.float64)
    den_g = np.zeros(N, dtype=np.float64)
    ar = np.arange
    for c in range(N_CORES):
        rowout = results[c]["rowout"].astype(np.float64)
        cs = results[c]["cs"].astype(np.float64)
        for b in range(VIEW):
            for i in range(2):
                m = 2 * b + i
                g = BATCH * b + (ROT * c + 128 * i + ar(128)) % BATCH
                num_g[g] += rowout[:, m] + rowout[:, 6 + m]
                den_g[g] += rowout[:, 12 + m] + rowout[:, 18 + m]
                if b == 0 and i == 0:
                    num_g[g] += rowout[:, 32]
                    den_g[g] += rowout[:, 33]
                pi_of = {p[0]: j for j, p in enumerate(PAIRS)}
                pi = pi_of[b]
                if pi == 2 and 2 * PAIRS[pi][0] + i == 2 * PAIRS[2][0] + 1:
                    den_g[g] += rowout[:, 30] + rowout[:, 31]
                else:
                    den_g[g] += rowout[:, 24 + pi * 2 + i]
                for cc in range(7):
                    gc = BATCH * b + (ROT * c + 128 * (i + 1 + cc) + ar(128)) % BATCH
                    num_g[gc] += cs[:, 14 * m + cc]
                    den_g[gc] += cs[:, 14 * m + 7 + cc]
        for pi, (_b1, b2) in enumerate(PAIRS):
            for i in range(2):
                idx = pi * 2 + i
                for cc in range(16):
                    gc = BATCH * b2 + (ROT * c + 128 * cc + ar(128)) % BATCH
                    den_g[gc] += cs[:, 84 + 16 * idx + cc]
    loss = -(np.log(num_g / den_g).sum() / BATCH)
    return np.float32(loss)


def run(features: np.ndarray, trace: bool = False):
    """Run the SPMD kernel; returns (loss ndarray, BassKernelResults)."""
    if "nc" not in _cache:
        _cache["nc"] = _build_nc()
    nc = _cache["nc"]
    in_maps = _prep_inputs(np.asarray(features))
    res = run_bass_kernel_spmd(nc, in_maps, core_ids=list(range(N_CORES)),
                               trace=trace)
    loss = _combine(res.results)
    return np.asarray(loss, dtype=np.float32), res


def kernel(features: np.ndarray) -> np.ndarray:
    loss, _ = run(features, trace=False)
    return loss
